# revision 7
# baseline (speedup 1.0000x reference)
"""Trainium2 Bass kernel for nn_BuildPatches (retrieval KNN).

Full inputs -> full outputs. Internally: batch-parallel across 8 NeuronCores,
one batch per core. Per core, for each 128-root block:
  PE computes cross = roots . points^T (K=3 fp32 matmul, bit-identical to the
  XLA einsum), ACT evicts fl(2*cross) - r0, DVE adds -r1 giving nd = -dist
  (bit-exact negation of the reference distance), DVE max8 over 64 chunks of
  256 builds a 512-candidate pool, 4 rounds of max8+match_replace select the
  top-32, max_index against the full row recovers global indices, and GPSIMD
  indirect DMA gathers the neighbor coordinates.
"""
import numpy as np

_CACHE = {}

B, N, R, P = 8, 16384, 2048, 32
NBLK = R // 128  # 16
CHUNK = 512  # matmul/psum chunk
NCH = N // CHUNK  # 32
SELCH = 256  # selection chunk for stage-1 max8
NSEL = N // SELCH  # 64


def _build_program():
    import concourse.bass as bass
    import concourse.mybir as mybir
    import concourse.tile as tile
    from concourse import bacc

    nc = bacc.Bacc("TRN2", target_bir_lowering=False, debug=False, num_devices=8)

    pts_d = nc.dram_tensor("points", [N, 3], mybir.dt.float32, kind="ExternalInput")
    roots_d = nc.dram_tensor("roots", [R, 3], mybir.dt.float32, kind="ExternalInput")
    dist_d = nc.dram_tensor("dist", [R, N], mybir.dt.float32, kind="ExternalOutput")
    patches_d = nc.dram_tensor("patches", [R, P, 3], mybir.dt.float32, kind="ExternalOutput")
    idx_d = nc.dram_tensor("idx", [R, P], mybir.dt.uint32, kind="ExternalOutput")
    pdist_d = nc.dram_tensor("pdist", [R, P], mybir.dt.float32, kind="ExternalOutput")
    pn2_scratch = nc.dram_tensor("pn2_scratch", [128, 128], mybir.dt.float32)

    f32 = mybir.dt.float32
    AT = mybir.ActivationFunctionType
    OP = mybir.AluOpType

    with tile.TileContext(nc) as tc:
        with tc.tile_pool(name="r1pool", bufs=1) as r1pool:
            r1repl = r1pool.tile([128, N], f32)  # replicated pn2 row, 64KB/part

            # ---------- setup: build r1repl = ||p||^2 replicated ----------
            with tc.tile_pool(name="setup", bufs=1) as sp:
                pts_nat = sp.tile([128, 128, 3], f32)
                nc.sync.dma_start(out=pts_nat[:], in_=pts_d[:].rearrange("(a b) d -> a b d", a=128))
                sq_nat = sp.tile([128, 128, 3], f32)
                nc.vector.tensor_tensor(out=sq_nat[:], in0=pts_nat[:], in1=pts_nat[:], op=OP.mult)
                pn2_nat = sp.tile([128, 128], f32)
                # fl(fl(x^2+y^2)+z^2) explicit order
                nc.vector.tensor_tensor(out=pn2_nat[:], in0=sq_nat[:, :, 0], in1=sq_nat[:, :, 1], op=OP.add)
                nc.vector.tensor_tensor(out=pn2_nat[:], in0=pn2_nat[:], in1=sq_nat[:, :, 2], op=OP.add)
                # roundtrip through DRAM: pn2_nat row-major == pn2 ordered by n
                nc.sync.dma_start(out=pn2_scratch[:], in_=pn2_nat[:])
                # replicate the flat [16384] row into every partition
                flat = pn2_scratch[:].rearrange("a b -> (a b)")
                for q in range(128):
                    nc.sync.dma_start(out=r1repl[q : q + 1, :], in_=flat[None, :])

            with (
                tc.tile_pool(name="main", bufs=2) as mp,
                tc.tile_pool(name="ndpool", bufs=2) as ndp,
                tc.tile_pool(name="stpool", bufs=2) as stp,
                tc.tile_pool(name="candpool", bufs=1) as cp,
                tc.tile_pool(name="ptspool", bufs=2) as pp,
                tc.tile_pool(name="psum", bufs=4, space="PSUM") as psum,
            ):
                for blk in range(NBLK):
                    r_lo = blk * 128
                    # roots for this block, natural layout [128, 3]
                    t_roots = mp.tile([128, 3], f32, tag="roots")
                    nc.sync.dma_start(out=t_roots[:], in_=roots_d[r_lo : r_lo + 128, :])
                    # transposed roots [3, 128] for matmul lhsT
                    t_rootsT = mp.tile([3, 128], f32, tag="rootsT")
                    nc.sync.dma_start(
                        out=t_rootsT[:], in_=roots_d[r_lo : r_lo + 128, :].rearrange("n d -> d n")
                    )
                    # r0 = fl(fl(rx^2+ry^2)+rz^2); neg_r0 = -r0  (bias for evict)
                    t_rsq = mp.tile([128, 3], f32, tag="rsq")
                    nc.vector.tensor_tensor(out=t_rsq[:], in0=t_roots[:], in1=t_roots[:], op=OP.mult)
                    t_nr0 = mp.tile([128, 1], f32, tag="nr0")
                    nc.vector.tensor_tensor(out=t_nr0[:], in0=t_rsq[:, 0:1], in1=t_rsq[:, 1:2], op=OP.add)
                    nc.vector.tensor_tensor(out=t_nr0[:], in0=t_nr0[:], in1=t_rsq[:, 2:3], op=OP.add)
                    nc.vector.tensor_scalar(
                        out=t_nr0[:], in0=t_nr0[:], scalar1=-1.0, scalar2=None, op0=OP.mult
                    )

                    t_nd = ndp.tile([128, N], f32, tag="nd")
                    for c in range(NCH):
                        t_pts = pp.tile([3, CHUNK], f32, tag="ptsch")
                        nc.sync.dma_start(
                            out=t_pts[:],
                            in_=pts_d[c * CHUNK : (c + 1) * CHUNK, :].rearrange("n d -> d n"),
                        )
                        pt = psum.tile([128, CHUNK], f32, space="PSUM")
                        nc.tensor.matmul(
                            out=pt[:], lhsT=t_rootsT[:], rhs=t_pts[:], start=True, stop=True
                        )
                        # evict: fl(2*cross) + (-r0)   (double-rounded scale-then-bias)
                        nc.scalar.activation(
                            out=t_nd[:, c * CHUNK : (c + 1) * CHUNK],
                            in_=pt[:],
                            func=AT.Identity,
                            bias=t_nr0[:],
                            scale=2.0,
                        )
                    # nd = (2c - r0) - r1  -> bit-exact -(reference dist)
                    nc.vector.tensor_tensor(out=t_nd[:], in0=t_nd[:], in1=r1repl[:], op=OP.subtract)

                    # dist out = -nd, staged in pieces, DMA'd per piece
                    for s in range(32):
                        t_st = stp.tile([128, 512], f32, tag="stage")
                        nc.vector.tensor_scalar(
                            out=t_st[:], in0=t_nd[:, s * 512 : (s + 1) * 512],
                            scalar1=-1.0, scalar2=None, op0=OP.mult,
                        )
                        nc.sync.dma_start(
                            out=dist_d[r_lo : r_lo + 128, s * 512 : (s + 1) * 512],
                            in_=t_st[:],
                        )

                    # stage-1: per-chunk top-8 candidates
                    t_cand = cp.tile([128, NSEL * 8], f32, tag="cand")
                    nd3 = t_nd[:].rearrange("p (c k) -> p c k", k=SELCH)
                    for c in range(NSEL):
                        nc.vector.max(out=t_cand[:, 8 * c : 8 * c + 8], in_=nd3[:, c])
                    # stage-2: top-32 of candidates (in-place match_replace)
                    t_top = mp.tile([128, P], f32, tag="top")
                    for r_ in range(4):
                        nc.vector.max(out=t_top[:, 8 * r_ : 8 * r_ + 8], in_=t_cand[:])
                        nc.vector.match_replace(
                            out=t_cand[:], in_to_replace=t_top[:, 8 * r_ : 8 * r_ + 8],
                            in_values=t_cand[:], imm_value=-3.0e38,
                        )
                    # winner global indices: max_index against the full row
                    t_wg = mp.tile([128, P], f32, tag="wgf")  # placeholder for alignment
                    t_widx = mp.tile([128, P], mybir.dt.uint32, tag="widx")
                    for r_ in range(4):
                        nc.vector.max_index(
                            out=t_widx[:, 8 * r_ : 8 * r_ + 8],
                            in_max=t_top[:, 8 * r_ : 8 * r_ + 8],
                            in_values=t_nd[:],
                        )
                    nc.sync.dma_start(out=idx_d[r_lo : r_lo + 128, :], in_=t_widx[:])
                    # sq_patches_dist = -top32
                    nc.vector.tensor_scalar(
                        out=t_wg[:], in0=t_top[:], scalar1=-1.0, scalar2=None, op0=OP.mult
                    )
                    nc.sync.dma_start(out=pdist_d[r_lo : r_lo + 128, :], in_=t_wg[:])

                    # gather neighbor coords: 32 indirect DMAs of [128,1]
                    t_praw = mp.tile([128, P, 3], f32, tag="praw")
                    for k in range(P):
                        nc.gpsimd.indirect_dma_start(
                            out=t_praw[:, k, :],
                            out_offset=None,
                            in_=pts_d[:],
                            in_offset=bass.IndirectOffsetOnAxis(ap=t_widx[:, k : k + 1], axis=0),
                        )
                    # patches = gathered - root (broadcast over P)
                    t_patch = mp.tile([128, P, 3], f32, tag="patch")
                    for d in range(3):
                        nc.vector.tensor_tensor(
                            out=t_patch[:, :, d],
                            in0=t_praw[:, :, d],
                            in1=t_roots[:, d : d + 1].to_broadcast([128, P]),
                            op=OP.subtract,
                        )
                    nc.sync.dma_start(out=patches_d[r_lo : r_lo + 128, :, :], in_=t_patch[:])
    nc.compile()
    return nc


def _get_program():
    if "nc" not in _CACHE:
        _CACHE["nc"] = _build_program()
    return _CACHE["nc"]


def kernel(points_pl, roots, patch_size):
    assert int(patch_size) == P
    points_pl = np.ascontiguousarray(np.asarray(points_pl, dtype=np.float32))
    roots = np.ascontiguousarray(np.asarray(roots, dtype=np.float32))
    assert points_pl.shape == (B, N, 3) and roots.shape == (B, R, 3)

    from concourse.bass_utils import run_bass_kernel_spmd

    nc = _get_program()
    in_maps = [{"points": points_pl[b], "roots": roots[b]} for b in range(B)]
    res = run_bass_kernel_spmd(nc, in_maps, list(range(B))).results

    sq_distance_mat = np.stack([res[b]["dist"] for b in range(B)])
    patches = np.stack([res[b]["patches"] for b in range(B)])
    idx = np.stack([res[b]["idx"] for b in range(B)]).astype(np.int32)
    sq_patches_dist = np.stack([res[b]["pdist"] for b in range(B)])

    # Host fix-up for rows where equal values straddle a group-of-8 boundary:
    # max_index then returns the same index twice. Detect duplicate indices per
    # row and recompute those rows exactly from the (bit-exact) distance matrix.
    idx_sorted = np.sort(idx, axis=2)
    bad = (np.diff(idx_sorted, axis=2) == 0).any(axis=2)  # [B, R]
    if bad.any():
        bb, rr = np.nonzero(bad)
        for b, r in zip(bb, rr):
            d = sq_distance_mat[b, r]
            order = np.argsort(d, kind="stable")[:P].astype(np.int32)
            idx[b, r] = order
            sq_patches_dist[b, r] = d[order]
            patches[b, r] = points_pl[b, order] - roots[b, r]

    batch_idx = np.broadcast_to(np.arange(B, dtype=np.int32)[:, None, None], (B, R, P))
    patches_idx = np.stack([batch_idx, idx], axis=-1)
    return patches, patches_idx, sq_patches_dist, sq_distance_mat


# revision 9
# speedup vs baseline: 1.8916x; 1.8916x over previous
"""Trainium2 Bass kernel for nn_BuildPatches (retrieval KNN).

Full inputs -> full outputs. Internally: batch-parallel across 8 NeuronCores,
one batch per core. Per core, for each 128-root block:
  PE computes cross = roots . points^T (K=3 fp32 matmul, bit-identical to the
  XLA einsum), ACT evicts fl(2*cross) - r0, DVE adds -r1 giving nd = -dist
  (bit-exact negation of the reference distance), DVE max8 over 64 chunks of
  256 builds a 512-candidate pool, 4 rounds of max8+match_replace select the
  top-32, max_index against the full row recovers global indices, and GPSIMD
  indirect DMA gathers the neighbor coordinates.
"""
import numpy as np

_CACHE = {}

B, N, R, P = 8, 16384, 2048, 32
NBLK = R // 128  # 16
CHUNK = 512  # matmul/psum chunk
NCH = N // CHUNK  # 32
SELCH = 256  # selection chunk for stage-1 max8
NSEL = N // SELCH  # 64


def _build_program():
    import concourse.bass as bass
    import concourse.mybir as mybir
    import concourse.tile as tile
    from concourse import bacc

    nc = bacc.Bacc("TRN2", target_bir_lowering=False, debug=False, num_devices=8)

    pts_d = nc.dram_tensor("points", [N, 3], mybir.dt.float32, kind="ExternalInput")
    ptsT_d = nc.dram_tensor("pointsT", [3, N], mybir.dt.float32, kind="ExternalInput")
    roots_d = nc.dram_tensor("roots", [R, 3], mybir.dt.float32, kind="ExternalInput")
    rootsT_d = nc.dram_tensor("rootsT", [3, R], mybir.dt.float32, kind="ExternalInput")
    dist_d = nc.dram_tensor("dist", [R, N], mybir.dt.float32, kind="ExternalOutput")
    patches_d = nc.dram_tensor("patches", [R, P, 3], mybir.dt.float32, kind="ExternalOutput")
    idx_d = nc.dram_tensor("idx", [R, P], mybir.dt.uint32, kind="ExternalOutput")
    pdist_d = nc.dram_tensor("pdist", [R, P], mybir.dt.float32, kind="ExternalOutput")
    pn2_scratch = nc.dram_tensor("pn2_scratch", [128, 128], mybir.dt.float32)

    f32 = mybir.dt.float32
    AT = mybir.ActivationFunctionType
    OP = mybir.AluOpType

    with tile.TileContext(nc) as tc:
        with tc.tile_pool(name="r1pool", bufs=1) as r1pool:
            r1repl = r1pool.tile([128, N], f32)  # replicated pn2 row, 64KB/part

            # ---------- setup: build r1repl = ||p||^2 replicated ----------
            with tc.tile_pool(name="setup", bufs=1) as sp:
                pts_nat = sp.tile([128, 128, 3], f32)
                nc.sync.dma_start(out=pts_nat[:], in_=pts_d[:].rearrange("(a b) d -> a b d", a=128))
                sq_nat = sp.tile([128, 128, 3], f32)
                nc.vector.tensor_tensor(out=sq_nat[:], in0=pts_nat[:], in1=pts_nat[:], op=OP.mult)
                pn2_nat = sp.tile([128, 128], f32)
                # fl(fl(x^2+y^2)+z^2) explicit order
                nc.vector.tensor_tensor(out=pn2_nat[:], in0=sq_nat[:, :, 0], in1=sq_nat[:, :, 1], op=OP.add)
                nc.vector.tensor_tensor(out=pn2_nat[:], in0=pn2_nat[:], in1=sq_nat[:, :, 2], op=OP.add)
                # roundtrip through DRAM: pn2_nat row-major == pn2 ordered by n
                nc.sync.dma_start(out=pn2_scratch[:], in_=pn2_nat[:])
                # replicate the flat [16384] row into every partition
                flat = pn2_scratch[:].rearrange("a b -> (a b)")
                for q in range(128):
                    nc.sync.dma_start(out=r1repl[q : q + 1, :], in_=flat[None, :])

            with (
                tc.tile_pool(name="main", bufs=2) as mp,
                tc.tile_pool(name="ndpool", bufs=2) as ndp,
                tc.tile_pool(name="stpool", bufs=2) as stp,
                tc.tile_pool(name="candpool", bufs=1) as cp,
                tc.tile_pool(name="ptspool", bufs=2) as pp,
                tc.tile_pool(name="psum", bufs=4, space="PSUM") as psum,
            ):
                for blk in range(NBLK):
                    r_lo = blk * 128
                    # roots for this block, natural layout [128, 3]
                    t_roots = mp.tile([128, 3], f32, tag="roots")
                    nc.sync.dma_start(out=t_roots[:], in_=roots_d[r_lo : r_lo + 128, :])
                    # transposed roots [3, 128] for matmul lhsT
                    t_rootsT = mp.tile([3, 128], f32, tag="rootsT")
                    nc.sync.dma_start(out=t_rootsT[:], in_=rootsT_d[:, r_lo : r_lo + 128])
                    # r0 = fl(fl(rx^2+ry^2)+rz^2); neg_r0 = -r0  (bias for evict)
                    t_rsq = mp.tile([128, 3], f32, tag="rsq")
                    nc.vector.tensor_tensor(out=t_rsq[:], in0=t_roots[:], in1=t_roots[:], op=OP.mult)
                    t_nr0 = mp.tile([128, 1], f32, tag="nr0")
                    nc.vector.tensor_tensor(out=t_nr0[:], in0=t_rsq[:, 0:1], in1=t_rsq[:, 1:2], op=OP.add)
                    nc.vector.tensor_tensor(out=t_nr0[:], in0=t_nr0[:], in1=t_rsq[:, 2:3], op=OP.add)
                    nc.vector.tensor_scalar(
                        out=t_nr0[:], in0=t_nr0[:], scalar1=-1.0, scalar2=None, op0=OP.mult
                    )

                    t_nd = ndp.tile([128, N], f32, tag="nd")
                    for c in range(NCH):
                        t_pts = pp.tile([3, CHUNK], f32, tag="ptsch")
                        nc.sync.dma_start(
                            out=t_pts[:], in_=ptsT_d[:, c * CHUNK : (c + 1) * CHUNK]
                        )
                        pt = psum.tile([128, CHUNK], f32, space="PSUM")
                        nc.tensor.matmul(
                            out=pt[:], lhsT=t_rootsT[:], rhs=t_pts[:], start=True, stop=True
                        )
                        # evict: fl(2*cross) + (-r0)   (double-rounded scale-then-bias)
                        nc.scalar.activation(
                            out=t_nd[:, c * CHUNK : (c + 1) * CHUNK],
                            in_=pt[:],
                            func=AT.Identity,
                            bias=t_nr0[:],
                            scale=2.0,
                        )
                    # nd = (2c - r0) - r1  -> bit-exact -(reference dist)
                    nc.vector.tensor_tensor(out=t_nd[:], in0=t_nd[:], in1=r1repl[:], op=OP.subtract)

                    # dist out = -nd, staged in pieces, DMA'd per piece
                    for s in range(32):
                        t_st = stp.tile([128, 512], f32, tag="stage")
                        if s % 2 == 0:
                            nc.vector.tensor_scalar(
                                out=t_st[:], in0=t_nd[:, s * 512 : (s + 1) * 512],
                                scalar1=-1.0, scalar2=None, op0=OP.mult,
                            )
                        else:
                            nc.scalar.activation(
                                out=t_st[:], in_=t_nd[:, s * 512 : (s + 1) * 512],
                                func=AT.Copy, scale=-1.0,
                            )
                        nc.sync.dma_start(
                            out=dist_d[r_lo : r_lo + 128, s * 512 : (s + 1) * 512],
                            in_=t_st[:],
                        )

                    # stage-1: per-chunk top-8 candidates
                    t_cand = cp.tile([128, NSEL * 8], f32, tag="cand")
                    nd3 = t_nd[:].rearrange("p (c k) -> p c k", k=SELCH)
                    for c in range(NSEL):
                        nc.vector.max(out=t_cand[:, 8 * c : 8 * c + 8], in_=nd3[:, c])
                    # stage-2: top-32 of candidates (in-place match_replace)
                    t_top = mp.tile([128, P], f32, tag="top")
                    for r_ in range(4):
                        nc.vector.max(out=t_top[:, 8 * r_ : 8 * r_ + 8], in_=t_cand[:])
                        nc.vector.match_replace(
                            out=t_cand[:], in_to_replace=t_top[:, 8 * r_ : 8 * r_ + 8],
                            in_values=t_cand[:], imm_value=-3.0e38,
                        )
                    # winner global indices: max_index against the full row
                    t_wg = mp.tile([128, P], f32, tag="wgf")  # placeholder for alignment
                    t_widx = mp.tile([128, P], mybir.dt.uint32, tag="widx")
                    for r_ in range(4):
                        nc.vector.max_index(
                            out=t_widx[:, 8 * r_ : 8 * r_ + 8],
                            in_max=t_top[:, 8 * r_ : 8 * r_ + 8],
                            in_values=t_nd[:],
                        )
                    nc.sync.dma_start(out=idx_d[r_lo : r_lo + 128, :], in_=t_widx[:])
                    # sq_patches_dist = -top32
                    nc.vector.tensor_scalar(
                        out=t_wg[:], in0=t_top[:], scalar1=-1.0, scalar2=None, op0=OP.mult
                    )
                    nc.sync.dma_start(out=pdist_d[r_lo : r_lo + 128, :], in_=t_wg[:])

                    # gather neighbor coords: 32 indirect DMAs of [128,1]
                    t_praw = mp.tile([128, P, 3], f32, tag="praw")
                    for k in range(P):
                        nc.gpsimd.indirect_dma_start(
                            out=t_praw[:, k, :],
                            out_offset=None,
                            in_=pts_d[:],
                            in_offset=bass.IndirectOffsetOnAxis(ap=t_widx[:, k : k + 1], axis=0),
                        )
                    # patches = gathered - root (broadcast over P)
                    t_patch = mp.tile([128, P, 3], f32, tag="patch")
                    for d in range(3):
                        nc.vector.tensor_tensor(
                            out=t_patch[:, :, d],
                            in0=t_praw[:, :, d],
                            in1=t_roots[:, d : d + 1].to_broadcast([128, P]),
                            op=OP.subtract,
                        )
                    nc.sync.dma_start(out=patches_d[r_lo : r_lo + 128, :, :], in_=t_patch[:])
    nc.compile()
    return nc


def _get_program():
    if "nc" not in _CACHE:
        _CACHE["nc"] = _build_program()
    return _CACHE["nc"]


def kernel(points_pl, roots, patch_size):
    assert int(patch_size) == P
    points_pl = np.ascontiguousarray(np.asarray(points_pl, dtype=np.float32))
    roots = np.ascontiguousarray(np.asarray(roots, dtype=np.float32))
    assert points_pl.shape == (B, N, 3) and roots.shape == (B, R, 3)

    from concourse.bass_utils import run_bass_kernel_spmd

    nc = _get_program()
    in_maps = [
        {
            "points": points_pl[b],
            "pointsT": np.ascontiguousarray(points_pl[b].T),
            "roots": roots[b],
            "rootsT": np.ascontiguousarray(roots[b].T),
        }
        for b in range(B)
    ]
    res = run_bass_kernel_spmd(nc, in_maps, list(range(B))).results

    sq_distance_mat = np.stack([res[b]["dist"] for b in range(B)])
    patches = np.stack([res[b]["patches"] for b in range(B)])
    idx = np.stack([res[b]["idx"] for b in range(B)]).astype(np.int32)
    sq_patches_dist = np.stack([res[b]["pdist"] for b in range(B)])

    # Host fix-up for rows where equal values straddle a group-of-8 boundary:
    # max_index then returns the same index twice. Detect duplicate indices per
    # row and recompute those rows exactly from the (bit-exact) distance matrix.
    idx_sorted = np.sort(idx, axis=2)
    bad = (np.diff(idx_sorted, axis=2) == 0).any(axis=2)  # [B, R]
    if bad.any():
        bb, rr = np.nonzero(bad)
        for b, r in zip(bb, rr):
            d = sq_distance_mat[b, r]
            order = np.argsort(d, kind="stable")[:P].astype(np.int32)
            idx[b, r] = order
            sq_patches_dist[b, r] = d[order]
            patches[b, r] = points_pl[b, order] - roots[b, r]

    batch_idx = np.broadcast_to(np.arange(B, dtype=np.int32)[:, None, None], (B, R, P))
    patches_idx = np.stack([batch_idx, idx], axis=-1)
    return patches, patches_idx, sq_patches_dist, sq_distance_mat
